# revision 3
# baseline (speedup 1.0000x reference)
"""GenerativeInfoNCE loss on 8 Trainium2 NeuronCores (Bass/Tile).

Data-parallel over batch per the sharding hint: 4 batches/core -> 2044 rows,
16 row-tiles of 128; negatives fetched with per-negative SWDGE indirect
row-gathers (128 x 2KB descriptors per instruction) from the full bf16 event
table in each core's HBM; per-row dot products on the Vector engine via
scalar_tensor_tensor with fused fp32 row-sum; per-row losses reduced on host
("psum-mean").  DVE op count is minimized:
  - |pred|^2, |pos|^2 row-norms move to the Scalar engine (activation Square
    with fused row-sum) — DVE keeps only the 10 negative dots + pred*pos.
  - The per-tile positive-score epilogue (norm mult, sqrt, reciprocal,
    scale) is batched across all 16 tiles into one pass of [128, NT] ops.
  - Gathers pipeline deeper (32 in flight) so Pool descriptor generation
    streams back-to-back under the DVE dots.
"""

import numpy as np

import concourse.bacc as bacc
import concourse.bass as bass
import concourse.tile as tile
from concourse import mybir
from concourse.bass_utils import run_bass_kernel_spmd

B, S, H, NEG = 32, 512, 1024, 10
NCORES = 8
BPC = B // NCORES            # batches per core
R = BPC * (S - 1)            # valid rows per core (2044)
NT = (R + 127) // 128        # 16 tiles of 128 rows
RP = NT * 128                # padded rows (2048)
K = NEG + 1                  # logits per row

BF16 = mybir.dt.bfloat16
F32 = mybir.dt.float32
I32 = mybir.dt.int32


def _build(temp: float, reps: int = 1):
    nc = bacc.Bacc("TRN2", target_bir_lowering=False, debug=False,
                   num_devices=NCORES)

    ev_d = nc.dram_tensor("events", [B * S, H], BF16, kind="ExternalInput")
    pred_d = nc.dram_tensor("pred", [RP, H], BF16, kind="ExternalInput")
    pos_d = nc.dram_tensor("pos", [RP, H], BF16, kind="ExternalInput")
    idx_d = nc.dram_tensor("idx", [128, NT * NEG], I32, kind="ExternalInput")
    out_d = nc.dram_tensor("loss", [128, NT], F32, kind="ExternalOutput")

    inv_t = 1.0 / temp
    mult = mybir.AluOpType.mult
    AF = mybir.ActivationFunctionType
    X = mybir.AxisListType.X

    with tile.TileContext(nc) as tc:
        import contextlib
        with contextlib.ExitStack() as ctx:
            io = ctx.enter_context(tc.tile_pool(name="io", bufs=4))
            gp = ctx.enter_context(tc.tile_pool(name="gather", bufs=32))
            scrp = ctx.enter_context(tc.tile_pool(name="scratch", bufs=2))
            pers = ctx.enter_context(tc.tile_pool(name="persist", bufs=1))

            loop_cm = tc.For_i(0, reps, 1) if reps > 1 else None
            if loop_cm is not None:
                ctx.enter_context(loop_cm)

            idx_t = pers.tile([128, NT * NEG], I32, tag="idx")
            nc.sync.dma_start(out=idx_t[:], in_=idx_d.ap())
            logits = pers.tile([128, NT * K], F32, tag="logits")
            pn2s = pers.tile([128, NT], F32, tag="pn2s")
            qn2s = pers.tile([128, NT], F32, tag="qn2s")
            ppds = pers.tile([128, NT], F32, tag="ppds")

            for t in range(NT):
                rs = slice(t * 128, (t + 1) * 128)
                pred_t = io.tile([128, H], BF16, tag="pred")
                nc.sync.dma_start(out=pred_t[:], in_=pred_d.ap()[rs, :])
                pos_t = io.tile([128, H], BF16, tag="pos")
                nc.sync.dma_start(out=pos_t[:], in_=pos_d.ap()[rs, :])

                scr = scrp.tile([128, H], BF16, tag="scr")
                ascr = scrp.tile([128, H], BF16, tag="ascr")
                for j in range(NEG):
                    gj = gp.tile([128, H], BF16, tag="g")
                    nc.gpsimd.indirect_dma_start(
                        out=gj[:],
                        out_offset=None,
                        in_=ev_d.ap(),
                        in_offset=bass.IndirectOffsetOnAxis(
                            ap=idx_t[:, t * NEG + j:t * NEG + j + 1],
                            axis=0),
                    )
                    c = t * K + 1 + j
                    nc.vector.scalar_tensor_tensor(
                        out=scr[:], in0=pred_t[:], scalar=inv_t,
                        in1=gj[:], op0=mult, op1=mult,
                        accum_out=logits[:, c:c + 1],
                    )

                nc.scalar.activation(
                    out=ascr[:], in_=pred_t[:], func=AF.Square,
                    accum_out=pn2s[:, t:t + 1])
                nc.scalar.activation(
                    out=ascr[:], in_=pos_t[:], func=AF.Square,
                    accum_out=qn2s[:, t:t + 1])
                nc.vector.scalar_tensor_tensor(
                    out=scr[:], in0=pred_t[:], scalar=inv_t, in1=pos_t[:],
                    op0=mult, op1=mult, accum_out=ppds[:, t:t + 1])

            # batched positive-score epilogue over all NT tiles
            l3 = logits[:].rearrange("p (t k) -> p t k", k=K)
            nrm = pers.tile([128, NT], F32, tag="nrm")
            nc.vector.tensor_mul(out=nrm[:], in0=pn2s[:], in1=qn2s[:])
            nc.scalar.activation(out=nrm[:], in_=nrm[:], func=AF.Sqrt)
            rn = pers.tile([128, NT], F32, tag="rn")
            nc.vector.reciprocal(out=rn[:], in_=nrm[:])
            nc.vector.tensor_mul(out=l3[:, :, 0], in0=ppds[:], in1=rn[:])

            # batched logsumexp over all NT tiles
            m = pers.tile([128, NT, 1], F32, tag="m")
            nc.vector.reduce_max(out=m[:], in_=l3, axis=X)
            sh = pers.tile([128, NT, K], F32, tag="sh")
            nc.vector.tensor_sub(out=sh[:], in0=l3, in1=m[:].to_broadcast([128, NT, K]))
            eh = pers.tile([128, NT, K], F32, tag="eh")
            nc.scalar.activation(out=eh[:], in_=sh[:], func=AF.Exp)
            ss = pers.tile([128, NT, 1], F32, tag="ss")
            nc.vector.reduce_sum(out=ss[:], in_=eh[:], axis=X)
            nc.scalar.activation(out=ss[:], in_=ss[:], func=AF.Ln)
            outt = pers.tile([128, NT], F32, tag="outt")
            nc.vector.tensor_add(out=outt[:], in0=m[:, :, 0], in1=ss[:, :, 0])
            nc.vector.tensor_sub(out=outt[:], in0=outt[:], in1=l3[:, :, 0])
            nc.sync.dma_start(out=out_d.ap(), in_=outt[:])

    nc.compile()
    return nc


def _prep_in_maps(encoder_outputs, event_embeddings, neg_indices):
    enc = np.asarray(encoder_outputs, dtype=np.float32)
    ev = np.asarray(event_embeddings, dtype=np.float32)
    ni = np.asarray(neg_indices)
    bf = mybir.dt.np(BF16)

    b_ids = np.arange(B, dtype=ni.dtype)[:, None, None]
    gidx = (ni + S * (ni >= b_ids * S).astype(ni.dtype)).astype(np.int32)

    ev_flat = np.ascontiguousarray(ev.reshape(B * S, H)).astype(bf)

    in_maps = []
    for c in range(NCORES):
        bs = slice(c * BPC, (c + 1) * BPC)
        pred = enc[bs, :-1, :].reshape(R, H)
        pos = ev[bs, 1:, :].reshape(R, H)
        pred_p = np.ones((RP, H), np.float32)
        pred_p[:R] = pred
        pos_p = np.ones((RP, H), np.float32)
        pos_p[:R] = pos
        idx = np.zeros((RP, NEG), np.int32)
        idx[:R] = gidx[bs].reshape(R, NEG)
        idx_pt = np.ascontiguousarray(
            idx.reshape(NT, 128, NEG).transpose(1, 0, 2)).reshape(128, NT * NEG)
        in_maps.append({
            "events": ev_flat,
            "pred": pred_p.astype(bf),
            "pos": pos_p.astype(bf),
            "idx": idx_pt,
        })
    return in_maps


def _reduce_loss(results) -> np.float32:
    total = 0.0
    for c in range(NCORES):
        lr = np.asarray(results[c]["loss"], dtype=np.float64)  # [128, NT]
        rows = lr.T.reshape(RP)[:R]
        total += rows.sum()
    return np.float32(total / (B * (S - 1)))


_NC_CACHE: dict = {}


def kernel(encoder_outputs, event_embeddings, neg_indices, temperature):
    temp = float(np.asarray(temperature))
    nc = _NC_CACHE.get(temp)
    if nc is None:
        nc = _build(temp)
        _NC_CACHE[temp] = nc
    in_maps = _prep_in_maps(encoder_outputs, event_embeddings, neg_indices)
    res = run_bass_kernel_spmd(nc, in_maps, core_ids=list(range(NCORES)))
    return _reduce_loss(res.results)


# revision 5
# speedup vs baseline: 1.0025x; 1.0025x over previous
"""GenerativeInfoNCE loss on 8 Trainium2 NeuronCores (Bass/Tile).

Data-parallel over batch per the sharding hint: 4 batches/core -> 2044 rows,
16 row-tiles of 128; negatives fetched with per-negative SWDGE indirect
row-gathers (128 x 2KB descriptors per instruction) from the full bf16 event
table in each core's HBM; per-row dot products on the Vector engine via
scalar_tensor_tensor with fused fp32 row-sum; per-row losses reduced on host
("psum-mean").  DVE op count is minimized:
  - |pred|^2, |pos|^2 row-norms move to the Scalar engine (activation Square
    with fused row-sum) — DVE keeps only the 10 negative dots + pred*pos.
  - The per-tile positive-score epilogue (norm mult, sqrt, reciprocal,
    scale) is batched across all 16 tiles into one pass of [128, NT] ops.
  - Gathers pipeline deeper (32 in flight) so Pool descriptor generation
    streams back-to-back under the DVE dots.
"""

import numpy as np

import concourse.bacc as bacc
import concourse.bass as bass
import concourse.tile as tile
from concourse import mybir
from concourse.bass_utils import run_bass_kernel_spmd

B, S, H, NEG = 32, 512, 1024, 10
NCORES = 8
BPC = B // NCORES            # batches per core
R = BPC * (S - 1)            # valid rows per core (2044)
NT = (R + 127) // 128        # 16 tiles of 128 rows
RP = NT * 128                # padded rows (2048)
K = NEG + 1                  # logits per row

BF16 = mybir.dt.bfloat16
F32 = mybir.dt.float32
I32 = mybir.dt.int32


def _build(temp: float, reps: int = 1):
    nc = bacc.Bacc("TRN2", target_bir_lowering=False, debug=False,
                   num_devices=NCORES)

    ev_d = nc.dram_tensor("events", [B * S, H], BF16, kind="ExternalInput")
    pred_d = nc.dram_tensor("pred", [RP, H], BF16, kind="ExternalInput")
    pos_d = nc.dram_tensor("pos", [RP, H], BF16, kind="ExternalInput")
    idx_d = nc.dram_tensor("idx", [128, NT * NEG], I32, kind="ExternalInput")
    out_d = nc.dram_tensor("loss", [128, NT], F32, kind="ExternalOutput")

    inv_t = 1.0 / temp
    mult = mybir.AluOpType.mult
    AF = mybir.ActivationFunctionType
    X = mybir.AxisListType.X

    with tile.TileContext(nc) as tc:
        import contextlib
        with contextlib.ExitStack() as ctx:
            io = ctx.enter_context(tc.tile_pool(name="io", bufs=4))
            gp = ctx.enter_context(tc.tile_pool(name="gather", bufs=32))
            scrp = ctx.enter_context(tc.tile_pool(name="scratch", bufs=2))
            pers = ctx.enter_context(tc.tile_pool(name="persist", bufs=1))

            loop_cm = tc.For_i(0, reps, 1) if reps > 1 else None
            if loop_cm is not None:
                ctx.enter_context(loop_cm)

            idx_t = pers.tile([128, NT * NEG], I32, tag="idx")
            nc.sync.dma_start(out=idx_t[:], in_=idx_d.ap())
            logits = pers.tile([128, NT * K], F32, tag="logits")
            pn2s = pers.tile([128, NT], F32, tag="pn2s")
            qn2s = pers.tile([128, NT], F32, tag="qn2s")
            ppds = pers.tile([128, NT], F32, tag="ppds")

            for t in range(NT):
                rs = slice(t * 128, (t + 1) * 128)
                pred_t = io.tile([128, H], BF16, tag="pred")
                nc.sync.dma_start(out=pred_t[:], in_=pred_d.ap()[rs, :])
                pos_t = io.tile([128, H], BF16, tag="pos")
                nc.sync.dma_start(out=pos_t[:], in_=pos_d.ap()[rs, :])

                scr = scrp.tile([128, H], BF16, tag="scr")
                ascr = scrp.tile([128, H], BF16, tag="ascr")
                for j in range(NEG):
                    gj = gp.tile([128, H], BF16, tag="g")
                    nc.gpsimd.indirect_dma_start(
                        out=gj[:],
                        out_offset=None,
                        in_=ev_d.ap(),
                        in_offset=bass.IndirectOffsetOnAxis(
                            ap=idx_t[:, t * NEG + j:t * NEG + j + 1],
                            axis=0),
                    )
                    c = t * K + 1 + j
                    nc.vector.scalar_tensor_tensor(
                        out=scr[:], in0=pred_t[:], scalar=inv_t,
                        in1=gj[:], op0=mult, op1=mult,
                        accum_out=logits[:, c:c + 1],
                    )

                nc.scalar.activation(
                    out=ascr[:], in_=pred_t[:], func=AF.Square,
                    accum_out=pn2s[:, t:t + 1])
                nc.scalar.activation(
                    out=ascr[:], in_=pos_t[:], func=AF.Square,
                    accum_out=qn2s[:, t:t + 1])
                nc.vector.scalar_tensor_tensor(
                    out=scr[:], in0=pred_t[:], scalar=inv_t, in1=pos_t[:],
                    op0=mult, op1=mult, accum_out=ppds[:, t:t + 1])

            # batched positive-score epilogue over all NT tiles
            l3 = logits[:].rearrange("p (t k) -> p t k", k=K)
            nrm = pers.tile([128, NT], F32, tag="nrm")
            nc.vector.tensor_mul(out=nrm[:], in0=pn2s[:], in1=qn2s[:])
            nc.scalar.activation(out=nrm[:], in_=nrm[:], func=AF.Sqrt)
            rn = pers.tile([128, NT], F32, tag="rn")
            nc.vector.reciprocal(out=rn[:], in_=nrm[:])
            nc.vector.tensor_mul(out=l3[:, :, 0], in0=ppds[:], in1=rn[:])

            # batched logsumexp over all NT tiles
            m = pers.tile([128, NT, 1], F32, tag="m")
            nc.vector.reduce_max(out=m[:], in_=l3, axis=X)
            sh = pers.tile([128, NT, K], F32, tag="sh")
            nc.vector.tensor_sub(out=sh[:], in0=l3, in1=m[:].to_broadcast([128, NT, K]))
            eh = pers.tile([128, NT, K], F32, tag="eh")
            nc.scalar.activation(out=eh[:], in_=sh[:], func=AF.Exp)
            ss = pers.tile([128, NT, 1], F32, tag="ss")
            nc.vector.reduce_sum(out=ss[:], in_=eh[:], axis=X)
            nc.scalar.activation(out=ss[:], in_=ss[:], func=AF.Ln)
            outt = pers.tile([128, NT], F32, tag="outt")
            nc.vector.tensor_add(out=outt[:], in0=m[:, :, 0], in1=ss[:, :, 0])
            nc.vector.tensor_sub(out=outt[:], in0=outt[:], in1=l3[:, :, 0])
            nc.sync.dma_start(out=out_d.ap(), in_=outt[:])

    nc.compile()
    return nc


def _prep_in_maps(encoder_outputs, event_embeddings, neg_indices):
    enc = np.asarray(encoder_outputs, dtype=np.float32)
    ev = np.asarray(event_embeddings, dtype=np.float32)
    ni = np.asarray(neg_indices)
    bf = mybir.dt.np(BF16)

    b_ids = np.arange(B, dtype=ni.dtype)[:, None, None]
    gidx = (ni + S * (ni >= b_ids * S).astype(ni.dtype)).astype(np.int32)

    ev_flat = np.ascontiguousarray(ev.reshape(B * S, H)).astype(bf)

    ev3 = ev_flat.reshape(B, S, H)  # pos rows are event rows: slice the
    enc_bf = enc.astype(bf)         # already-cast bf16 tables, no second
    in_maps = []                    # cast and no f32 staging copies
    for c in range(NCORES):
        bs = slice(c * BPC, (c + 1) * BPC)
        pred_p = np.ones((RP, H), bf)
        pred_p[:R] = enc_bf[bs, :-1, :].reshape(R, H)
        pos_p = np.ones((RP, H), bf)
        pos_p[:R] = ev3[bs, 1:, :].reshape(R, H)
        idx = np.zeros((RP, NEG), np.int32)
        idx[:R] = gidx[bs].reshape(R, NEG)
        idx_pt = np.ascontiguousarray(
            idx.reshape(NT, 128, NEG).transpose(1, 0, 2)).reshape(128, NT * NEG)
        in_maps.append({
            "events": ev_flat,
            "pred": pred_p,
            "pos": pos_p,
            "idx": idx_pt,
        })
    return in_maps


def _reduce_loss(results) -> np.float32:
    total = 0.0
    for c in range(NCORES):
        lr = np.asarray(results[c]["loss"], dtype=np.float64)  # [128, NT]
        rows = lr.T.reshape(RP)[:R]
        total += rows.sum()
    return np.float32(total / (B * (S - 1)))


_NC_CACHE: dict = {}


def kernel(encoder_outputs, event_embeddings, neg_indices, temperature):
    temp = float(np.asarray(temperature))
    nc = _NC_CACHE.get(temp)
    if nc is None:
        nc = _build(temp)
        _NC_CACHE[temp] = nc
    in_maps = _prep_in_maps(encoder_outputs, event_embeddings, neg_indices)
    res = run_bass_kernel_spmd(nc, in_maps, core_ids=list(range(NCORES)))
    return _reduce_loss(res.results)
